# revision 17
# baseline (speedup 1.0000x reference)
"""Bidirectional tanh-RNN (B=64, T=2048, I=64, H=128, O=64) on 8 trn2 cores.

Strategy: time-parallel chunked recurrence with warmup.
  - Each core owns a 256-wide time window and runs BOTH directions.
  - Each direction's window is split into G=8 sub-chunks of L=32 steps that
    run in lockstep as one macro-chain of width G*B=512 columns.
  - Each sub-chunk starts from h=0 and runs W=28 warmup steps on real inputs
    before its window; the tanh RNN contracts (~0.7x/step), so the hidden
    state converges to the exact trajectory to ~the fp32r noise floor.
  - x is zero-padded outside [0,T) including the bias ("ones") row, so
    global-boundary chunks stay exactly at h=0 through the pad (exact).
  - All matmuls run as float32r (single-pass fp32 on the PE, 4x faster than
    full fp32); measured end-to-end relative error ~3e-4.

Per macro-step j (per direction d):
  z = Waug_d.T @ xaug(j)         (matmul, K=65: 64 x rows + ones row, PSUM)
  z += W_hh_d.T.T @ h(j-1)       (matmul accumulate)
  h(j) = tanh(z)                 (ScalarE, PSUM -> SBUF)
  if j >= W:  o_d = W_out_half_d.T.T @ h(j) (PSUM); DVE adds into out slab
              (first writer adds b_out via tensor_scalar); DMA out when
              both directions have contributed to a column group.
"""

import os
import sys

sys.path.insert(0, "/opt/trn_rl_repo")

import numpy as np

import concourse.bass as bass
import concourse.mybir as mybir
from concourse import bacc
from concourse.tile import TileContext

N_CORES = 8
B, T, I, H, O = 64, 2048, 64, 128, 64
FP = mybir.dt.float32


class Cfg:
    def __init__(self, B=64, T=2048, I=64, H=128, O=64, n_cores=8, G=8, W=16,
                 XCH=None, mm_f32r=True, v2=True, zbufs=3, obufs=1,
                 packw=0, xahead=2, v3=0, v4=0, bf16=1, eldw=0):
        self.B, self.T, self.I, self.H, self.O = B, T, I, H, O
        self.n_cores = n_cores
        self.TWIN = T // n_cores          # per-core time window
        self.G = G                        # sub-chunks per direction
        self.L = self.TWIN // G           # useful steps per sub-chunk
        self.W = W                        # warmup steps
        self.NSTEP = self.L + W           # macro-steps per chain
        self.HALO = max(16, W) if v4 else W   # x slab halo (fixed for v4 so
                                              # XW stays DMA-chunk friendly)
        self.XW = self.TWIN + 2 * self.HALO   # x slab width (t slots)
        if XCH is None:
            XCH = next(c for c in (16, 8, 4, 2) if self.XW % c == 0)
        self.XCH = XCH                    # x slab DMA chunk width (t slots)
        self.mm_f32r = mm_f32r            # run matmuls in float32r (1 cyc/row)
        self.v2 = v2                      # packed PSUM->DRAM out path
        self.zbufs = zbufs                # PSUM z tile rotation depth
        self.obufs = obufs                # PSUM out tile rotation depth
        self.packw = packw                # matmul lhsT direct from packed tile
        self.xahead = xahead              # steps the x-matmuls run ahead
        self.v3 = v3                      # v1 structure + fast startup
        self.v4 = v4                      # packed-o psum tile + bf16 path
        self.bf16 = bf16                  # (v4) bf16 weights/x/h
        self.eldw = eldw                  # (v4, bf16) explicit ldweights
        if v3:
            self.v2 = False
        if v4:
            self.v2 = False
            self.v3 = 0
            if obufs == 1:
                self.obufs = 2   # packed-o frees a PSUM bank; avoid the
                                 # o-matmul-behind-DVE stall
        self.NB = G * B                   # macro-step column width
        self.KI = I + 1                   # x rows + ones row
        assert self.XW % XCH == 0, (self.XW, XCH)
        assert self.L % 2 == 0


def x_first_need(cfg: Cfg):
    """first_need[chunk_idx] = earliest macro-step j that reads any t-slot in
    the chunk, over both directions and all sub-chunks."""
    G, L, W, NSTEP = cfg.G, cfg.L, cfg.W, cfg.NSTEP
    nchunks = cfg.XW // cfg.XCH
    first = [NSTEP] * nchunks
    for j in range(NSTEP):
        for g in range(G):
            for i in (g * L + j, (g + 1) * L - 1 + 2 * W - j):
                ci = i // cfg.XCH
                if first[ci] > j:
                    first[ci] = j
    return sorted(range(nchunks), key=lambda ci: first[ci])


def build_nc(cfg: Cfg, nrep=1, loop_n=None):
    nc = bacc.Bacc()
    G, L, W, NSTEP, NB, KI = cfg.G, cfg.L, cfg.W, cfg.NSTEP, cfg.NB, cfg.KI
    B, Hh, Oo = cfg.B, cfg.H, cfg.O

    FPR = mybir.dt.float32r if cfg.mm_f32r else FP

    xc = nc.dram_tensor("xc", [KI, cfg.XW, B], FPR, kind="ExternalInput")
    wih_f = nc.dram_tensor("wih_f", [KI, Hh], FPR, kind="ExternalInput")
    wih_b = nc.dram_tensor("wih_b", [KI, Hh], FPR, kind="ExternalInput")
    whh_f = nc.dram_tensor("whh_f", [Hh, Hh], FPR, kind="ExternalInput")
    whh_b = nc.dram_tensor("whh_b", [Hh, Hh], FPR, kind="ExternalInput")
    wof = nc.dram_tensor("wof", [Hh, Oo], FPR, kind="ExternalInput")
    wob = nc.dram_tensor("wob", [Hh, Oo], FPR, kind="ExternalInput")
    bo = nc.dram_tensor("bo", [Oo, 1], FP, kind="ExternalInput")
    outT = nc.dram_tensor("outT", [Oo, L, G, B], FP, kind="ExternalOutput")

    with TileContext(nc) as tc:
        with (
            tc.tile_pool(name="singles", bufs=1) as singles,
            tc.tile_pool(name="zps", bufs=3, space="PSUM") as zpool,
            tc.tile_pool(name="ops", bufs=1, space="PSUM") as opool,
        ):
            # tiny dummy tanh up front so the ACT table load overlaps x DMA
            dummy = singles.tile([1, 1], FP, tag="dummy")
            nc.gpsimd.memset(dummy[:], 0.0)
            nc.scalar.activation(
                dummy[:], dummy[:], mybir.ActivationFunctionType.Tanh
            )

            # --- weights into SBUF (once) ---
            s_wih = [
                singles.tile([KI, Hh], FPR, tag=f"wih{d}", name=f"wih{d}")
                for d in range(2)
            ]
            s_whh = [
                singles.tile([Hh, Hh], FPR, tag=f"whh{d}", name=f"whh{d}")
                for d in range(2)
            ]
            s_wo = [
                singles.tile([Hh, Oo], FPR, tag=f"wo{d}", name=f"wo{d}")
                for d in range(2)
            ]
            s_bo = singles.tile([Oo, 1], FP, tag="bo")

            def emit_weight_dmas():
                nc.sync.dma_start(s_wih[0][:], wih_f[:])
                nc.gpsimd.dma_start(s_wih[1][:], wih_b[:])
                nc.sync.dma_start(s_whh[0][:], whh_f[:])
                nc.gpsimd.dma_start(s_whh[1][:], whh_b[:])
                nc.sync.dma_start(s_wo[0][:], wof[:])
                nc.gpsimd.dma_start(s_wo[1][:], wob[:])
                nc.sync.dma_start(s_bo[:], bo[:])

            # h scratch (ping-pong) per direction
            scr = [
                [
                    singles.tile([Hh, NB], FPR, tag=f"scr{d}{p}", name=f"scr{d}{p}")
                    for p in range(3)
                ]
                for d in range(2)
            ]
            # output slab (64 partitions, TWIN*B columns viewed as (G, L, B))
            slab = singles.tile([Oo, L, G, B], FP, tag="slab")

            # x slab, DMA'd in first-need order
            xs = singles.tile([KI, cfg.XW, B], FPR, tag="xs")

            from contextlib import nullcontext

            loop_ctx = (
                tc.For_i(0, loop_n, 1) if loop_n is not None else nullcontext()
            )
            with loop_ctx:
             for _it in range(nrep):
              plan = x_first_need(cfg)
              kw = min(15, len(plan) - 1)
              for k, ci in enumerate(plan):
                c0 = ci * cfg.XCH
                # alternate HWDGE (sync) / SWDGE (gpsimd) queues so DMA
                # descriptor generation for the startup fetch runs in parallel
                eng = nc.sync if k % 2 == 0 else nc.gpsimd
                eng.dma_start(
                    xs[:, c0 : c0 + cfg.XCH, :], xc[:, c0 : c0 + cfg.XCH, :]
                )
                if k == kw and _it == 0:
                    # weights go behind the step-0 x chunks: their descriptor
                    # generation no longer delays the startup-critical fetch
                    emit_weight_dmas()

              def emit_out(j, d):
                  # out-projection for step j (emitted one step late so the
                  # PE FIFO isn't blocked behind ACT(j))
                  zo = opool.tile([Oo, NB], FP, tag=f"o{d}", name=f"o{d}_{j}")
                  nc.tensor.matmul(
                      zo[:], s_wo[d][:], scr[d][j % 3][:], start=True, stop=True
                  )
                  r = (j - W) if d == 0 else (L - 1 - (j - W))
                  first = (r <= L // 2 - 1) if d == 0 else (r >= L // 2)
                  slab_ap = slab[:, r, :, :]
                  if first:
                      nc.vector.tensor_scalar_add(slab_ap, zo[:], s_bo[:, 0:1])
                  else:
                      nc.vector.tensor_add(slab_ap, slab_ap, zo[:])
                      nc.sync.dma_start(outT[:, r, :, :], slab_ap)

              for j in range(NSTEP):
                for d in range(2):
                    if d == 0:
                        i0 = j
                    else:
                        i0 = L + 2 * W - 1 - j
                    x_ap = xs[:, i0 : i0 + (G - 1) * L + 1 : L, :]
                    z = zpool.tile([Hh, NB], FP, tag=f"z{d}")
                    nc.tensor.matmul(
                        z[:],
                        s_wih[d][:],
                        x_ap,
                        start=True,
                        stop=(j == 0),
                    )
                    if j > 0:
                        nc.tensor.matmul(
                            z[:],
                            s_whh[d][:],
                            scr[d][(j - 1) % 3][:],
                            start=False,
                            stop=True,
                        )
                    h_cur = scr[d][j % 3]
                    nc.scalar.activation(
                        h_cur[:], z[:], mybir.ActivationFunctionType.Tanh
                    )
                for d in range(2):
                    if j - 1 >= W:
                        emit_out(j - 1, d)
              for d in range(2):
                  emit_out(NSTEP - 1, d)
    return nc


def build_nc_v3(cfg: Cfg, nrep=1, loop_n=None):
    """v1 structure (slab out path, same emission order) with only the
    startup path reworked: weight DMAs split across both queues up front in
    first-use order, a contiguous host-prepped xpre covering the slots steps
    0..7 read (5 small DMAs), and the big-slab chunk plan ordered by first
    need from step 8 on."""
    nc = bacc.Bacc()
    G, L, W, NSTEP, NB, KI = cfg.G, cfg.L, cfg.W, cfg.NSTEP, cfg.NB, cfg.KI
    B, Hh, Oo = cfg.B, cfg.H, cfg.O
    PF = 8
    SW = L + 2 * W

    FPR = mybir.dt.float32r if cfg.mm_f32r else FP

    xc = nc.dram_tensor("xc", [KI, cfg.XW, B], FPR, kind="ExternalInput")
    xpre = nc.dram_tensor("xpre", [KI, PF, 2, G, B], FPR, kind="ExternalInput")
    wih_f = nc.dram_tensor("wih_f", [KI, Hh], FPR, kind="ExternalInput")
    wih_b = nc.dram_tensor("wih_b", [KI, Hh], FPR, kind="ExternalInput")
    whh_f = nc.dram_tensor("whh_f", [Hh, Hh], FPR, kind="ExternalInput")
    whh_b = nc.dram_tensor("whh_b", [Hh, Hh], FPR, kind="ExternalInput")
    wof = nc.dram_tensor("wof", [Hh, Oo], FPR, kind="ExternalInput")
    wob = nc.dram_tensor("wob", [Hh, Oo], FPR, kind="ExternalInput")
    bo = nc.dram_tensor("bo", [Oo, 1], FP, kind="ExternalInput")
    outT = nc.dram_tensor("outT", [Oo, L, G, B], FP, kind="ExternalOutput")

    with TileContext(nc) as tc:
        with (
            tc.tile_pool(name="singles", bufs=1) as singles,
            tc.tile_pool(name="zps", bufs=3, space="PSUM") as zpool,
            tc.tile_pool(name="ops", bufs=1, space="PSUM") as opool,
        ):
            # tiny dummy tanh up front so the ACT table load overlaps x DMA
            dummy = singles.tile([1, 1], FP, tag="dummy")
            nc.gpsimd.memset(dummy[:], 0.0)
            nc.scalar.activation(
                dummy[:], dummy[:], mybir.ActivationFunctionType.Tanh
            )

            s_wih = [
                singles.tile([KI, Hh], FPR, tag=f"wih{d}", name=f"wih{d}")
                for d in range(2)
            ]
            s_whh = [
                singles.tile([Hh, Hh], FPR, tag=f"whh{d}", name=f"whh{d}")
                for d in range(2)
            ]
            s_wo = [
                singles.tile([Hh, Oo], FPR, tag=f"wo{d}", name=f"wo{d}")
                for d in range(2)
            ]
            s_bo = singles.tile([Oo, 1], FP, tag="bo")

            scr = [
                [
                    singles.tile([Hh, NB], FPR, tag=f"scr{d}{p}", name=f"scr{d}{p}")
                    for p in range(3)
                ]
                for d in range(2)
            ]
            slab = singles.tile([Oo, L, G, B], FP, tag="slab")
            xs = singles.tile([KI, cfg.XW, B], FPR, tag="xs")
            xpr = singles.tile([KI, PF, 2, G, B], FPR, tag="xpr")

            from contextlib import nullcontext

            loop_ctx = (
                tc.For_i(0, loop_n, 1) if loop_n is not None else nullcontext()
            )
            with loop_ctx:
             for _it in range(nrep):
              # startup order: what step 0 needs first, split across queues
              nc.sync.dma_start(s_wih[0][:], wih_f[:])
              nc.gpsimd.dma_start(s_wih[1][:], wih_b[:])
              nc.sync.dma_start(xpr[:, 0], xpre[:, 0])
              nc.gpsimd.dma_start(xpr[:, 1], xpre[:, 1])
              nc.sync.dma_start(s_whh[0][:], whh_f[:])
              nc.gpsimd.dma_start(s_whh[1][:], whh_b[:])
              nc.sync.dma_start(xpr[:, 2:4], xpre[:, 2:4])
              nc.gpsimd.dma_start(xpr[:, 4:6], xpre[:, 4:6])
              nc.sync.dma_start(xpr[:, 6:8], xpre[:, 6:8])
              nc.gpsimd.dma_start(s_wo[0][:], wof[:])
              nc.sync.dma_start(s_wo[1][:], wob[:])
              nc.gpsimd.dma_start(s_bo[:], bo[:])

              # big slab, first-need order for steps >= PF
              nchunks = cfg.XW // cfg.XCH

              def first_need(ci):
                  best = NSTEP
                  for j in range(PF, NSTEP):
                      for g in range(G):
                          for i in (g * L + j, g * L + SW - 1 - j):
                              if i // cfg.XCH == ci and j < best:
                                  best = j
                  return best

              plan = sorted(range(nchunks), key=first_need)
              for k, ci in enumerate(plan):
                c0 = ci * cfg.XCH
                eng = nc.sync if k % 2 == 0 else nc.gpsimd
                eng.dma_start(
                    xs[:, c0 : c0 + cfg.XCH, :], xc[:, c0 : c0 + cfg.XCH, :]
                )

              def emit_out(j, d):
                  # out-projection for step j (emitted one step late so the
                  # PE FIFO isn't blocked behind ACT(j))
                  zo = opool.tile([Oo, NB], FP, tag=f"o{d}", name=f"o{d}_{j}")
                  nc.tensor.matmul(
                      zo[:], s_wo[d][:], scr[d][j % 3][:], start=True, stop=True
                  )
                  r = (j - W) if d == 0 else (L - 1 - (j - W))
                  first = (r <= L // 2 - 1) if d == 0 else (r >= L // 2)
                  slab_ap = slab[:, r, :, :]
                  if first:
                      nc.vector.tensor_scalar_add(slab_ap, zo[:], s_bo[:, 0:1])
                  else:
                      nc.vector.tensor_add(slab_ap, slab_ap, zo[:])
                      nc.sync.dma_start(outT[:, r, :, :], slab_ap)

              for j in range(NSTEP):
                for d in range(2):
                    if j < PF:
                        x_ap = xpr[:, j, d, :, :]
                    else:
                        i0 = j if d == 0 else SW - 1 - j
                        x_ap = xs[:, i0 : i0 + (G - 1) * L + 1 : L, :]
                    z = zpool.tile([Hh, NB], FP, tag=f"z{d}")
                    nc.tensor.matmul(
                        z[:],
                        s_wih[d][:],
                        x_ap,
                        start=True,
                        stop=(j == 0),
                    )
                    if j > 0:
                        nc.tensor.matmul(
                            z[:],
                            s_whh[d][:],
                            scr[d][(j - 1) % 3][:],
                            start=False,
                            stop=True,
                        )
                    h_cur = scr[d][j % 3]
                    nc.scalar.activation(
                        h_cur[:], z[:], mybir.ActivationFunctionType.Tanh
                    )
                for d in range(2):
                    if j - 1 >= W:
                        emit_out(j - 1, d)
              for d in range(2):
                  emit_out(NSTEP - 1, d)
    return nc


def build_nc_v2(cfg: Cfg, nrep=1, loop_n=None):
    """v2 redesign (see v1 docstring for the algorithm):
      - out-projection pair packed into one [128, NB] PSUM tile (fwd rows
        0:64, bwd rows 64:128), one DVE bias-add to SBUF, one DMA per step;
        the f+b cross-block sum happens on the host during unshard.
      - all weights in one packed [128, 640] DRAM tensor -> single DMA on the
        sync queue; the scalar queue carries no DMAs so tanh(0) isn't stuck
        behind descriptor generation.
      - xpre: host-prepped contiguous copy of the x slots steps 0..7 read,
        fetched in 4 small DMAs so compute starts ~2us in while the big flat
        x slab streams in behind it (first-need order, XCH-slot chunks).
      - x-matmuls run two steps ahead of the recurrence and the PE emission
        order interleaves directions (h_f, x_f, out_f, h_b, x_b, out_b) so
        the PE queue stays dense and each tanh hides under the other
        direction's matmuls; separate z tiles per direction keep the Tile
        dependency tracker from coupling the two chains.
    """
    nc = bacc.Bacc()
    G, L, W, NSTEP, NB, KI = cfg.G, cfg.L, cfg.W, cfg.NSTEP, cfg.NB, cfg.KI
    B, Hh, Oo = cfg.B, cfg.H, cfg.O
    PF = 8                               # steps served by the xpre prefetch
    SW = L + 2 * W

    FPR = mybir.dt.float32r if cfg.mm_f32r else FP

    xc = nc.dram_tensor("xc", [KI, cfg.XW, B], FPR, kind="ExternalInput")
    xpre = nc.dram_tensor("xpre", [KI, PF, 2, G, B], FPR, kind="ExternalInput")
    wpack = nc.dram_tensor("wpack", [Hh, 640], FPR, kind="ExternalInput")
    bo = nc.dram_tensor("bo", [Oo, 1], FP, kind="ExternalInput")
    outT = nc.dram_tensor("outT2", [Oo, 2, L, G, B], FP, kind="ExternalOutput")

    with TileContext(nc) as tc:
        with (
            tc.tile_pool(name="singles", bufs=1) as singles,
            tc.tile_pool(name="osb", bufs=3) as opool_sb,
            tc.tile_pool(name="zps", bufs=cfg.zbufs, space="PSUM") as zpool,
            tc.tile_pool(name="ops", bufs=cfg.obufs, space="PSUM") as opool,
        ):
            # tiny dummy tanh up front so the ACT table load overlaps x DMA
            dummy = singles.tile([1, 1], FP, tag="dummy")
            nc.gpsimd.memset(dummy[:], 0.0)
            nc.scalar.activation(
                dummy[:], dummy[:], mybir.ActivationFunctionType.Tanh
            )

            s_bo = singles.tile([Oo, 1], FP, tag="bo")
            if cfg.packw:
                s_w = singles.tile([Hh, 640], FPR, tag="wpack")
                s_wih = [s_w[0:KI, 0:128], s_w[0:KI, 128:256]]
                s_whh = [s_w[:, 256:384], s_w[:, 384:512]]
                s_wo = [s_w[:, 512:576], s_w[:, 576:640]]
            else:
                # separate tiles so matmul lhsT APs are whole tensors (sliced
                # lhsT measured ~70ns/matmul slower on HW); still one DRAM
                # input, DMA'd a slice at a time in first-use order.
                s_wih_t = [
                    singles.tile([KI, Hh], FPR, tag=f"wih{d}", name=f"wih{d}")
                    for d in range(2)
                ]
                s_whh_t = [
                    singles.tile([Hh, Hh], FPR, tag=f"whh{d}", name=f"whh{d}")
                    for d in range(2)
                ]
                s_wo_t = [
                    singles.tile([Hh, Oo], FPR, tag=f"wo{d}", name=f"wo{d}")
                    for d in range(2)
                ]
                s_wih = [t[:] for t in s_wih_t]
                s_whh = [t[:] for t in s_whh_t]
                s_wo = [t[:] for t in s_wo_t]

            # h scratch (rotating) per direction
            scr = [
                [
                    singles.tile([Hh, NB], FPR, tag=f"scr{d}{p}", name=f"scr{d}{p}")
                    for p in range(3)
                ]
                for d in range(2)
            ]

            xs = singles.tile([KI, cfg.XW, B], FPR, tag="xs")
            xpr = singles.tile([KI, PF, 2, G, B], FPR, tag="xpr")

            from contextlib import nullcontext

            loop_ctx = (
                tc.For_i(0, loop_n, 1) if loop_n is not None else nullcontext()
            )
            with loop_ctx:
             for _it in range(nrep):
              if cfg.packw:
                  nc.sync.dma_start(s_w[:], wpack[:])
              else:
                  nc.sync.dma_start(s_wih_t[0][:], wpack[0:KI, 0:128])
                  nc.sync.dma_start(s_wih_t[1][:], wpack[0:KI, 128:256])
              # steps 0-1 per direction as 4 small DMAs (first gates x-mm(0))
              for j in (0, 1):
                  for d in (0, 1):
                      nc.sync.dma_start(
                          xpr[:, j, d], xpre[:, j, d]
                      )
              if not cfg.packw:
                  nc.sync.dma_start(s_whh_t[0][:], wpack[:, 256:384])
                  nc.sync.dma_start(s_whh_t[1][:], wpack[:, 384:512])
              nc.sync.dma_start(s_bo[:], bo[:])
              for k in range(1, PF // 2):
                  nc.sync.dma_start(
                      xpr[:, 2 * k : 2 * k + 2], xpre[:, 2 * k : 2 * k + 2]
                  )
              if not cfg.packw:
                  nc.sync.dma_start(s_wo_t[0][:], wpack[:, 512:576])
                  nc.sync.dma_start(s_wo_t[1][:], wpack[:, 576:640])
              # big slab in first-need order for steps >= PF, 2 queues
              nchunks = cfg.XW // cfg.XCH

              def first_need(ci):
                  best = NSTEP
                  for j in range(PF, NSTEP):
                      for g in range(G):
                          for i in (g * L + j, g * L + SW - 1 - j):
                              if i // cfg.XCH == ci and j < best:
                                  best = j
                  return best

              plan = sorted(range(nchunks), key=first_need)
              for k, ci in enumerate(plan):
                c0 = ci * cfg.XCH
                eng = nc.gpsimd if k % 2 == 0 else nc.sync
                eng.dma_start(
                    xs[:, c0 : c0 + cfg.XCH, :], xc[:, c0 : c0 + cfg.XCH, :]
                )

              def emit_x(j, d):
                  # x-projection for step j; opens the PSUM accumulation
                  z = zpool.tile([Hh, NB], FP, tag=f"z{d}", name=f"z{d}_{j}")
                  if j < PF:
                      x_ap = xpr[:, j, d, :, :]
                  else:
                      i0 = j if d == 0 else SW - 1 - j
                      x_ap = xs[:, i0 : i0 + (G - 1) * L + 1 : L, :]
                  nc.tensor.matmul(
                      z[:], s_wih[d], x_ap, start=True, stop=(j == 0)
                  )
                  return z

              def emit_out(j, d):
                  # out-projection for step j, direction d (one step late so
                  # the PE FIFO isn't blocked behind ACT(j)). PSUM can't be
                  # DMA'd directly; one DVE op per half moves it to SBUF (the
                  # fwd half adds b_out; the bwd half is a plain copy and the
                  # host adds the two halves, flipped, during unshard).
                  zo = opool.tile([Oo, NB], FP, tag=f"o{d}", name=f"o{d}_{j}")
                  nc.tensor.matmul(
                      zo[:], s_wo[d], scr[d][j % 3][:], start=True, stop=True
                  )
                  osb = opool_sb.tile(
                      [Oo, NB], FP, tag=f"osb{d}", name=f"osb{d}_{j}"
                  )
                  if d == 0:
                      nc.vector.tensor_scalar_add(osb[:], zo[:], s_bo[:, 0:1])
                      nc.sync.dma_start(outT[:, 0, j - W, :, :], osb[:])
                  else:
                      nc.vector.tensor_copy(osb[:], zo[:])
                      nc.gpsimd.dma_start(outT[:, 1, j - W, :, :], osb[:])

              A = cfg.xahead
              zs = {
                  (j, d): emit_x(j, d)
                  for j in range(max(A, 1))
                  for d in range(2)
              }
              for j in range(NSTEP):
                for d in range(2):
                    z = zs.pop((j, d)) if (j, d) in zs else emit_x(j, d)
                    if j > 0:
                        nc.tensor.matmul(
                            z[:],
                            s_whh[d],
                            scr[d][(j - 1) % 3][:],
                            start=False,
                            stop=True,
                        )
                    nc.scalar.activation(
                        scr[d][j % 3][:],
                        z[:],
                        mybir.ActivationFunctionType.Tanh,
                    )
                    if A > 0 and j + A < NSTEP:
                        zs[(j + A, d)] = emit_x(j + A, d)
                    if j - 1 >= W:
                        emit_out(j - 1, d)
              for d in range(2):
                  emit_out(NSTEP - 1, d)
    return nc


def build_nc_v4(cfg: Cfg, nrep=1, loop_n=None):
    """v2 redesign with a packed out PSUM tile and (optionally) bf16:
      - o_f and o_b land in ONE [128, NB] PSUM tile (fwd rows 0:64, bwd rows
        64:128, both at chain-local index j-W; host flips+sums during
        unshard). One bank per step instead of two -> obufs=2 fits in PSUM,
        so the out matmul no longer serializes behind the previous step's
        DVE copy (v2's obufs=1 cost ~550ns/step of PE stall).
      - one DVE bias-add [128, NB] and one DMA [128 part, NB] per step.
      - bf16 weights/x/h: FWL-accelerated LDWEIGHTS (f32r disables FWL and
        pays ~107ns/matmul on HW), halved x/out DMA and DVE bytes.
    """
    nc = bacc.Bacc()
    G, L, W, NSTEP, NB, KI = cfg.G, cfg.L, cfg.W, cfg.NSTEP, cfg.NB, cfg.KI
    B, Hh, Oo = cfg.B, cfg.H, cfg.O
    PF = 12                              # steps served by the xpre prefetch
    SW = L + 2 * W
    OFF = cfg.HALO - W                   # slab halo minus warmup offset

    DT = mybir.dt.bfloat16 if cfg.bf16 else (
        mybir.dt.float32r if cfg.mm_f32r else FP
    )

    xc = nc.dram_tensor("xc", [KI, cfg.XW, B], DT, kind="ExternalInput")
    xpre = nc.dram_tensor("xpre", [KI, PF, 2, G, B], DT, kind="ExternalInput")
    wpack = nc.dram_tensor("wpack", [Hh, 640], DT, kind="ExternalInput")
    bo = nc.dram_tensor("bo", [Hh, 1], FP, kind="ExternalInput")
    outT = nc.dram_tensor("outT4", [Hh, L, G, B], DT, kind="ExternalOutput")

    with TileContext(nc) as tc:
        with (
            tc.tile_pool(name="singles", bufs=1) as singles,
            tc.tile_pool(name="osb", bufs=3) as opool_sb,
            tc.tile_pool(name="zps", bufs=cfg.zbufs, space="PSUM") as zpool,
            tc.tile_pool(name="ops", bufs=cfg.obufs, space="PSUM") as opool,
        ):
            # tiny dummy tanh up front so the ACT table load overlaps x DMA
            dummy = singles.tile([1, 1], FP, tag="dummy")
            nc.gpsimd.memset(dummy[:], 0.0)
            nc.scalar.activation(
                dummy[:], dummy[:], mybir.ActivationFunctionType.Tanh
            )

            s_bo = singles.tile([Hh, 1], FP, tag="bo")
            s_wih_t = [
                singles.tile([KI, Hh], DT, tag=f"wih{d}", name=f"wih{d}")
                for d in range(2)
            ]
            s_whh_t = [
                singles.tile([Hh, Hh], DT, tag=f"whh{d}", name=f"whh{d}")
                for d in range(2)
            ]
            s_wo_t = [
                singles.tile([Hh, Oo], DT, tag=f"wo{d}", name=f"wo{d}")
                for d in range(2)
            ]
            s_wih = [t[:] for t in s_wih_t]
            s_whh = [t[:] for t in s_whh_t]
            s_wo = [t[:] for t in s_wo_t]

            scr = [
                [
                    singles.tile([Hh, NB], DT, tag=f"scr{d}{p}", name=f"scr{d}{p}")
                    for p in range(3)
                ]
                for d in range(2)
            ]

            xs = singles.tile([KI, cfg.XW, B], DT, tag="xs")
            xpr = singles.tile([KI, PF, 2, G, B], DT, tag="xpr")

            from contextlib import nullcontext

            loop_ctx = (
                tc.For_i(0, loop_n, 1) if loop_n is not None else nullcontext()
            )
            with loop_ctx:
             for _it in range(nrep):
              # big-slab chunk plan: first-need order for steps >= PF
              nchunks = cfg.XW // cfg.XCH

              def first_need(ci):
                  best = NSTEP
                  for j in range(PF, NSTEP):
                      for g in range(G):
                          for i in (
                              g * L + OFF + j,
                              g * L + OFF + SW - 1 - j,
                          ):
                              if i // cfg.XCH == ci and j < best:
                                  best = j
                  return best

              plan = sorted(range(nchunks), key=first_need)

              def slab_dma(eng, ci):
                  c0 = ci * cfg.XCH
                  eng.dma_start(
                      xs[:, c0 : c0 + cfg.XCH, :], xc[:, c0 : c0 + cfg.XCH, :]
                  )

              # gpsimd queue: the whole slab in first-need order (desc-gen
              # ~1us/chunk on the idle gpsimd engine, starts at t=0); the
              # sync queue then carries only weights + xpre + out slabs.
              for ci in plan:
                  slab_dma(nc.gpsimd, ci)
              nc.sync.dma_start(s_wih_t[0][:], wpack[0:KI, 0:128])
              nc.sync.dma_start(xpr[:, 0, 0], xpre[:, 0, 0])
              nc.sync.dma_start(s_wih_t[1][:], wpack[0:KI, 128:256])
              nc.sync.dma_start(xpr[:, 0, 1], xpre[:, 0, 1])
              nc.sync.dma_start(s_whh_t[0][:], wpack[:, 256:384])
              nc.sync.dma_start(s_whh_t[1][:], wpack[:, 384:512])
              nc.sync.dma_start(xpr[:, 1], xpre[:, 1])
              for k in range(1, PF // 2):
                  nc.sync.dma_start(
                      xpr[:, 2 * k : 2 * k + 2], xpre[:, 2 * k : 2 * k + 2]
                  )
              nc.sync.dma_start(s_wo_t[0][:], wpack[:, 512:576])
              nc.sync.dma_start(s_wo_t[1][:], wpack[:, 576:640])
              nc.sync.dma_start(s_bo[:], bo[:])

              def mm(out_ap, w_ap, rhs_ap, start, stop):
                  # explicit ldweights lets the PE reorder window prefetch
                  # the stationary operand into the background weight buffer
                  # while the previous matmul streams (bf16 only).
                  if cfg.eldw and cfg.bf16:
                      nc.tensor.ldweights(w_ap)
                  nc.tensor.matmul(
                      out_ap, w_ap, rhs_ap, start=start, stop=stop
                  )

              def emit_x(j, d):
                  # x-projection for step j; opens the PSUM accumulation
                  z = zpool.tile([Hh, NB], FP, tag=f"z{d}", name=f"z{d}_{j}")
                  if j < PF:
                      x_ap = xpr[:, j, d, :, :]
                  else:
                      i0 = OFF + (j if d == 0 else SW - 1 - j)
                      x_ap = xs[:, i0 : i0 + (G - 1) * L + 1 : L, :]
                  mm(z[:], s_wih[d], x_ap, start=True, stop=(j == 0))
                  return z

              def emit_omm(j, d, zo):
                  # out-projection for step j, direction d into the shared
                  # [128, NB] PSUM tile: fwd rows 0:64, bwd rows 64:128.
                  mm(
                      zo[64 * d : 64 * (d + 1), :],
                      s_wo[d],
                      scr[d][j % 3][:],
                      start=True,
                      stop=True,
                  )

              def emit_oflush(j, zo):
                  # both directions written: bias-add to SBUF, single DMA.
                  osb = opool_sb.tile([Hh, NB], DT, tag="osb", name=f"osb_{j}")
                  nc.vector.tensor_scalar_add(osb[:], zo[:], s_bo[:, 0:1])
                  nc.sync.dma_start(outT[:, j - W, :, :], osb[:])

              A = cfg.xahead
              zs = {
                  (j, d): emit_x(j, d)
                  for j in range(max(A, 1))
                  for d in range(2)
              }
              zos = {}
              for j in range(NSTEP):
                for d in range(2):
                    z = zs.pop((j, d)) if (j, d) in zs else emit_x(j, d)
                    if j > 0:
                        mm(
                            z[:],
                            s_whh[d],
                            scr[d][(j - 1) % 3][:],
                            start=False,
                            stop=True,
                        )
                    nc.scalar.activation(
                        scr[d][j % 3][:],
                        z[:],
                        mybir.ActivationFunctionType.Tanh,
                    )
                    if A > 0 and j + A < NSTEP:
                        zs[(j + A, d)] = emit_x(j + A, d)
                    if j - 1 >= W:
                        if d == 0:
                            zo = opool.tile(
                                [Hh, NB], FP, tag="o", name=f"o_{j - 1}"
                            )
                            zos[j - 1] = zo
                        emit_omm(j - 1, d, zos[j - 1])
                if j - 1 >= W:
                    emit_oflush(j - 1, zos.pop(j - 1))
              zo = opool.tile([Hh, NB], FP, tag="o", name=f"o_{NSTEP - 1}")
              for d in range(2):
                  emit_omm(NSTEP - 1, d, zo)
              emit_oflush(NSTEP - 1, zo)
    return nc


def _prep_core_inputs(cfg: Cfg, c, x, packs):
    """Build per-core input map. x: (B,T,I). packs: dict of shared weights."""
    W, B_, T_ = cfg.W, cfg.B, cfg.T
    KI = cfg.KI
    t0 = c * cfg.TWIN
    # base: (KI, XW, B) covering global t in [t0-HALO, t0+TWIN+HALO)
    base = np.zeros((KI, cfg.XW, B_), np.float32)
    lo = t0 - cfg.HALO
    src_lo, src_hi = max(0, lo), min(T_, lo + cfg.XW)
    if src_hi > src_lo:
        # x (B,T,I) -> (I, t, B)
        base[: cfg.I, src_lo - lo : src_hi - lo, :] = np.transpose(
            x[:, src_lo:src_hi, :], (2, 1, 0)
        )
        base[cfg.I, src_lo - lo : src_hi - lo, :] = 1.0
    m = dict(packs)
    if cfg.v4 and cfg.bf16:
        import ml_dtypes

        base = base.astype(ml_dtypes.bfloat16)
    m["xc"] = base
    if cfg.v2 or cfg.v3 or cfg.v4:
        # contiguous copy of the slots the first PF steps read (both dirs)
        PF = 12 if cfg.v4 else 8
        SW, OFF = cfg.L + 2 * W, cfg.HALO - W
        xpre = np.empty((KI, PF, 2, cfg.G, B_), base.dtype)
        for j in range(PF):
            for g in range(cfg.G):
                xpre[:, j, 0, g] = base[:, g * cfg.L + OFF + j]
                xpre[:, j, 1, g] = base[:, g * cfg.L + OFF + SW - 1 - j]
        m["xpre"] = xpre
    return m


def _prep_shared(cfg, W_ih_f, W_hh_f, b_ih_f, b_hh_f, W_ih_b, W_hh_b, b_ih_b, b_hh_b,
                 W_out, b_out):
    KI, H_, O_ = cfg.KI, cfg.H, cfg.O

    def aug(W_ih, b_ih, b_hh):
        w = np.zeros((KI, H_), np.float32)
        w[: cfg.I] = W_ih.T
        w[cfg.I] = b_ih + b_hh
        return w

    if cfg.v2 or cfg.v4:
        wpack = np.zeros((H_, 640), np.float32)
        wpack[:KI, 0:128] = aug(W_ih_f, b_ih_f, b_hh_f)
        wpack[:KI, 128:256] = aug(W_ih_b, b_ih_b, b_hh_b)
        wpack[:, 256:384] = W_hh_f.T
        wpack[:, 384:512] = W_hh_b.T
        wpack[:, 512:576] = W_out[:, :H_].T
        wpack[:, 576:640] = W_out[:, H_:].T
        if cfg.v4:
            # bias only on the fwd rows; bwd rows contribute 0 so the host
            # flip+sum keeps a single b_out per output element.
            bo = np.zeros((H_, 1), np.float32)
            bo[:O_, 0] = np.asarray(b_out, np.float32)
            if cfg.bf16:
                import ml_dtypes

                wpack = wpack.astype(ml_dtypes.bfloat16)
            return {"wpack": wpack, "bo": bo}
        bo = np.ascontiguousarray(np.asarray(b_out, np.float32)[:, None])
        return {"wpack": wpack, "bo": np.ascontiguousarray(bo)}
    return {
        "wih_f": aug(W_ih_f, b_ih_f, b_hh_f),
        "wih_b": aug(W_ih_b, b_ih_b, b_hh_b),
        "whh_f": np.ascontiguousarray(W_hh_f.T),
        "whh_b": np.ascontiguousarray(W_hh_b.T),
        "wof": np.ascontiguousarray(W_out[:, :H_].T),
        "wob": np.ascontiguousarray(W_out[:, H_:].T),
        "bo": np.ascontiguousarray(b_out[:, None]),
    }


_NC_CACHE = {}


def _build(cfg: Cfg, nrep=1, loop_n=None):
    fn = (
        build_nc_v4
        if cfg.v4
        else (build_nc_v3 if cfg.v3 else (build_nc_v2 if cfg.v2 else build_nc))
    )
    return fn(cfg, nrep=nrep, loop_n=loop_n)


def _unpack(cfg: Cfg, results, b_out):
    """results: list of per-core dicts. Returns full (B, T, O) output."""
    outs = []
    for c in range(cfg.n_cores):
        if cfg.v4:
            oc = results[c]["outT4"].astype(np.float32)
            oc = oc.reshape(2 * cfg.O, cfg.L, cfg.G, cfg.B)
            # rows 0:64 = fwd at r=k (bias included); rows 64:128 = bwd at
            # chain-local k (true r = L-1-k): flip and sum.
            core = oc[: cfg.O] + oc[cfg.O :, ::-1, :, :]
        elif cfg.v2:
            oc = results[c]["outT2"].reshape(cfg.O, 2, cfg.L, cfg.G, cfg.B)
            # [:, 0, k] = fwd contribution for r=k (bias included);
            # [:, 1, k] = bwd contribution for r = L-1-k: flip and sum.
            core = oc[:, 0] + oc[:, 1, ::-1, :, :]
        else:
            core = results[c]["outT"].reshape(cfg.O, cfg.L, cfg.G, cfg.B)
        outs.append(
            np.transpose(core, (0, 2, 1, 3)).reshape(cfg.O, cfg.TWIN, cfg.B)
        )
    outT = np.concatenate(outs, axis=1)  # (O, T, B)
    return np.ascontiguousarray(np.transpose(outT, (2, 1, 0)))  # (B, T, O)


def kernel(x, W_ih_f, W_hh_f, b_ih_f, b_hh_f, W_ih_b, W_hh_b, b_ih_b, b_hh_b,
           W_out, b_out, _trace=False, _cfg=None):
    from concourse.bass_utils import run_bass_kernel_spmd

    cfg = _cfg if _cfg is not None else Cfg()
    key = (cfg.G, cfg.W, cfg.XCH, cfg.mm_f32r, cfg.v2, cfg.v3, cfg.zbufs,
           cfg.obufs, cfg.packw, cfg.xahead, cfg.v4, cfg.bf16)
    if key not in _NC_CACHE:
        nc = _build(cfg)
        nc.finalize()
        _NC_CACHE[key] = nc
    nc = _NC_CACHE[key]

    packs = _prep_shared(
        cfg, W_ih_f, W_hh_f, b_ih_f, b_hh_f, W_ih_b, W_hh_b, b_ih_b, b_hh_b,
        W_out, b_out,
    )
    x = np.asarray(x, np.float32)
    in_maps = [_prep_core_inputs(cfg, c, x, packs) for c in range(cfg.n_cores)]
    res = run_bass_kernel_spmd(
        nc, in_maps, core_ids=list(range(cfg.n_cores)), trace=_trace
    )
    out = _unpack(cfg, res.results, np.asarray(b_out, np.float32))
    if _trace:
        return out, res
    return out

